# revision 44
# baseline (speedup 1.0000x reference)
"""Trainium2 Bass kernel for the masked-attention block (nn_MAB_61607010894006).

Sharding: data-parallel over batch B=8 across 8 NeuronCores (one batch row
per core, weights replicated, no collectives).

Per-core strategy: activations live transposed ("feature-major",
[features, tokens]); all matmul operands are float16 (full PE rate; the
fp32/fp32r path runs in fp32_mode=HIGH at half clock). PSUM accumulation is
fp32 throughout, evictions round to fp16.

  qT/kT      = W.T @ X.T      (lhsT = W chunk, rhs = XT chunk)
  S^T        = kT_h' @ qT_h   (k tokens on partitions, q tokens free)
  softmax    : exp on ScalarE with mask as per-partition bias (-1e9), no
               max-subtraction (scores are O(1)); normalization deferred:
  o^T        = [v | 1]' @ A^T accumulated over k tiles -> row 64 is the
               softmax denominator; the PSUM result is evicted to SBUF at
               once (freeing the bank for the next head), the denominator
               row inverted in place on the ACT table, PE-broadcast across
               the 64 head lanes, then normalize+residual on DVE. The
               broadcast+normalize of head h is emitted in the middle of
               head h+1's score matmuls so the PE never waits on the ACT
               reciprocal (table swap ~1.3us). Odd heads assemble at
               partition base 0 and are shifted to lanes 64..127 by a
               Pool-queue SBUF DMA (engine ops cannot cross bases).
  layernorm  : feature-dim (partition) sums via ones-column matmuls on PE;
               per-token stats broadcast back via tiny PE ones-matmuls.
  FC         = Wo' @ OT, bias+relu fused into the ScalarE eviction.

Mask compaction: only unmasked key tokens are shipped per core (masked ones
contribute exactly +0.0 to the softmax numerator and denominator), padded
to a 128 multiple.
"""

import sys

sys.path.insert(0, "/opt/trn_rl_repo")

import numpy as np

import concourse.bass as bass
import concourse.mybir as mybir
import concourse.tile as tile
from concourse.bass_utils import run_bass_kernel_spmd

F32 = mybir.dt.float32
F16 = mybir.dt.float16
AF = mybir.ActivationFunctionType

B, NQ, NK, D, H, DH = 8, 1024, 1024, 512, 8, 64
EPS = 1e-5
NEG = -1e9
N_CORES = 8

MM = F16


def _split_multi_waits(nc):
    """This toolchain's walrus allows ONE sem wait per TPB instruction; Tile
    can emit several (kernel-tail drain). Hoist extras onto preceding
    single-wait NOPs on the same engine stream (equivalent: in-order issue).
    """
    multi_update = []
    for fn in nc.m.functions:
        for bb in fn.blocks:
            insts = bb.instructions
            new = []
            changed = False
            for inst in insts:
                si = inst.sync_info
                if si is not None and si.on_wait and len(si.on_wait) > 1:
                    waits = list(si.on_wait)
                    for w in waits[:-1]:
                        nop = mybir.InstNoOp(
                            name=f"I-wsplit-{nc.next_id()}", engine=inst.engine
                        )
                        nop.sync_info = mybir.SyncInfo(on_wait=[w], on_update=[])
                        new.append(nop)
                    inst.sync_info = mybir.SyncInfo(
                        on_wait=[waits[-1]], on_update=list(si.on_update)
                    )
                    changed = True
                if si is not None and si.on_update and len(si.on_update) > 1:
                    multi_update.append(inst.name)
                new.append(inst)
            if changed:
                bb.instructions = new
    if multi_update:
        raise RuntimeError(f">1 sem update unsupported: {multi_update[:10]}")


def _act_raw(nc, out, in_, func, bias=0.0, scale=1.0):
    """Raw InstActivation (bypasses the bass Reciprocal/Rsqrt accuracy guard;
    measured ~4e-5 max rel err on HW, inside this kernel's error budget)."""
    eng = nc.scalar
    inputs = [eng.lower_ap(in_)]
    for arg in (bias, scale, 0.0):
        inputs.append(mybir.ImmediateValue(dtype=mybir.dt.float32, value=arg))
    return eng.add_instruction(
        mybir.InstActivation(
            name=f"I-actraw-{nc.next_id()}",
            func=func,
            ins=inputs,
            outs=[eng.lower_ap(out)],
        )
    )


def chunks(n, w=512):
    out, s = [], 0
    while s < n:
        out.append((s, min(w, n - s)))
        s += min(w, n - s)
    return out


def build_nc(kt_tiles=8, gb_trivial=False):
    NKP = kt_tiles * 128  # compacted+padded key/value token count
    nc = bass.Bass()

    qt_d = nc.dram_tensor("qt", [D, NQ], MM, kind="ExternalInput")
    kt_d = nc.dram_tensor("kt", [D + 1, NKP], MM, kind="ExternalInput")  # +ones
    wq_d = nc.dram_tensor("wq", [D, D], MM, kind="ExternalInput")
    wk_d = nc.dram_tensor("wk", [D, D], MM, kind="ExternalInput")
    wv_d = nc.dram_tensor("wv", [D + 1, D], MM, kind="ExternalInput")  # +bv row
    wo_d = nc.dram_tensor("wo", [D, D], MM, kind="ExternalInput")
    bq_d = nc.dram_tensor("bq", [128, 4], F32, kind="ExternalInput")
    bk_d = nc.dram_tensor("bk", [128, 4], F32, kind="ExternalInput")
    bo_d = nc.dram_tensor("bo", [128, 4], F32, kind="ExternalInput")
    mb_d = nc.dram_tensor("mb", [128, kt_tiles], F32, kind="ExternalInput")
    gb_d = nc.dram_tensor("gb", [128, 16], F32, kind="ExternalInput")  # g0b0g1b1
    on_d = nc.dram_tensor("on", [128, 128], MM, kind="ExternalInput")  # all ones
    out_d = nc.dram_tensor("out", [D, NQ], MM, kind="ExternalOutput")

    mult, add = mybir.AluOpType.mult, mybir.AluOpType.add

    with tile.TileContext(nc) as tc:
        with (
            tc.tile_pool(name="wp", bufs=1) as wp,
            tc.tile_pool(name="ap", bufs=1) as ap,
            tc.tile_pool(name="sm", bufs=2) as sm,
            tc.tile_pool(name="pp", bufs=2, space="PSUM") as pp,
        ):
            # ---- weights (one batched DMA each; wo deferred to the end of
            # the issue stream since it is only needed at phase 4) ----------
            wq_sb = wp.tile([128, 4 * D], MM, name="wq_sb")
            wk_sb = wp.tile([128, 4 * D], MM, name="wk_sb")
            wv_sb = wp.tile([128, 4 * D], MM, name="wv_sb")
            wv1_sb = wp.tile([1, D], MM, name="wv1_sb")
            wo_sb = wp.tile([128, 4 * D], MM, name="wo_sb")
            bq_sb = wp.tile([128, 4], F32, name="bq_sb")
            bk_sb = wp.tile([128, 4], F32, name="bk_sb")
            bo_sb = wp.tile([128, 4], F32, name="bo_sb")
            mb_sb = wp.tile([128, kt_tiles], F32, name="mb_sb")
            gb_sb = wp.tile([128, 16], F32, name="gb_sb")
            ones_sb = wp.tile([128, 128], MM, name="ones_sb")
            kt1_sb = wp.tile([1, NKP], MM, name="kt1_sb")

            def load4(dst, src, n):
                # dst [128, 4*n] <- src [4*128, n] as one strided DMA
                nc.sync.dma_start(
                    dst.rearrange("p (t n) -> p t n", t=4),
                    src.rearrange("(t p) n -> p t n", p=128),
                )

            # ---- staging (released after phase 1) ----------------------------
            with tc.tile_pool(name="stg", bufs=1) as stg:
                qt_sb = stg.tile([128, 4 * NQ], MM, name="qt_sb")
                kt_sb = stg.tile([128, 4 * NKP], MM, name="kt_sb")

                # single DMA engine services all queues: use ONE queue in
                # strict priority order (consumers first), qt/kt chunked so
                # the first projection starts after ~0.75MB
                nc.sync.dma_start(bq_sb[:], bq_d[:])
                # first operands split across all three DMA-capable queues so
                # descriptor generation and transfer overlap
                qs = [nc.sync, nc.scalar, nc.gpsimd]
                for hc in range(2):
                    for kc in range(4):
                        if hc == 0:
                            qs[kc % 3].dma_start(
                                wq_sb[:, kc * D : (kc + 1) * D],
                                wq_d[kc * 128 : (kc + 1) * 128, :],
                            )
                        qs[(kc * 2 + hc + 1) % 3].dma_start(
                            qt_sb[:, kc * NQ + hc * 512 : kc * NQ + (hc + 1) * 512],
                            qt_d[kc * 128 : (kc + 1) * 128,
                                 hc * 512 : (hc + 1) * 512],
                        )
                nc.sync.dma_start(bk_sb[:], bk_d[:])
                for kc in range(4):
                    nc.sync.dma_start(
                        wk_sb[:, kc * D : (kc + 1) * D],
                        wk_d[kc * 128 : (kc + 1) * 128, :],
                    )
                    nc.sync.dma_start(
                        kt_sb[:, kc * NKP : kc * NKP + 512],
                        kt_d[kc * 128 : (kc + 1) * 128, 0:512],
                    )
                for kc in range(4):
                    nc.sync.dma_start(
                        kt_sb[:, kc * NKP + 512 : (kc + 1) * NKP],
                        kt_d[kc * 128 : (kc + 1) * 128, 512:NKP],
                    )
                nc.sync.dma_start(
                    wv_sb.rearrange("p (t n) -> p t n", t=4),
                    wv_d[0:D, :].rearrange("(t p) n -> p t n", p=128),
                )
                nc.sync.dma_start(wv1_sb[:, :], wv_d[D : D + 1, :])
                nc.sync.dma_start(kt1_sb[:, :], kt_d[D : D + 1, :])
                nc.sync.dma_start(mb_sb[:], mb_d[:])
                nc.sync.dma_start(ones_sb[:], on_d[:])
                nc.sync.dma_start(gb_sb[:], gb_d[:])
                nc.sync.dma_start(bo_sb[:], bo_d[:])
                nc.sync.dma_start(
                    wo_sb.rearrange("p (t n) -> p t n", t=4),
                    wo_d.rearrange("(t p) n -> p t n", p=128),
                )

                ones128 = ones_sb[:, 0:1]
                ones_f32 = wp.tile([128, 1], F32, name="ones_f32")
                nc.vector.memset(ones_f32[:], 1.0)

                # ---- persistent activations ------------------------------
                # q kept as two parity copies with the other head's rows
                # zeroed: scores then contract over the full 128-row head
                # pair (zeros kill the other head), avoiding the PE's
                # 64-row row-group switching penalty
                q_ev = ap.tile([128, 4 * NQ], MM, name="q_ev")
                q_od = ap.tile([128, 4 * NQ], MM, name="q_od")
                nc.vector.memset(q_ev[64:128, :], 0.0)
                nc.vector.memset(q_od[0:64, :], 0.0)
                k_sb = ap.tile([128, 4 * NKP], MM, name="k_sb", tag="kmm_sq")
                # v: per k-tile, 8 heads of [v(64)|1]
                v_sb = ap.tile([128, kt_tiles * 520], MM, name="v_sb")
                v_ones = v_sb.rearrange(
                    "p (i hh x) -> p i hh x", i=kt_tiles, hh=8
                )[:, :, :, 64]
                nc.vector.memset(v_ones, 1.0)

                # ---- phase 1: projections (all Q first: the Q operands are
                # the first 1.5MB to land, K/V stream in behind) -----------
                for cs, cw in chunks(NQ):
                    for t in range(4):
                        ps_q = pp.tile([128, 512], F32, name="ps_q", tag="pp")
                        for kc in range(4):
                            nc.tensor.matmul(
                                ps_q[:, 0:cw],
                                wq_sb[:, kc * D + t * 128 : kc * D + (t + 1) * 128],
                                qt_sb[:, kc * NQ + cs : kc * NQ + cs + cw],
                                start=(kc == 0),
                                stop=(kc == 3),
                            )
                        dst = slice(t * NQ + cs, t * NQ + cs + cw)
                        nc.scalar.activation(
                            q_ev[0:64, dst], ps_q[0:64, 0:cw], AF.Identity,
                            bias=bq_sb[0:64, t : t + 1],
                        )
                        nc.scalar.activation(
                            q_od[64:128, dst], ps_q[64:128, 0:cw], AF.Identity,
                            bias=bq_sb[64:128, t : t + 1],
                        )
                for cs, cw in chunks(NKP):
                    for t in range(4):
                        ps_k = pp.tile([128, 512], F32, name="ps_k", tag="pp")
                        for kc in range(4):
                            nc.tensor.matmul(
                                ps_k[:, 0:cw],
                                wk_sb[:, kc * D + t * 128 : kc * D + (t + 1) * 128],
                                kt_sb[:, kc * NKP + cs : kc * NKP + cs + cw],
                                start=(kc == 0),
                                stop=(kc == 3),
                            )
                        dst = slice(t * NKP + cs, t * NKP + cs + cw)
                        nc.scalar.activation(
                            k_sb[:, dst], ps_k[:, 0:cw], AF.Identity,
                            bias=bk_sb[:, t : t + 1],
                        )

                # v token-major [NKP, 512] (+bias via augmented ones row)
                for vt in range(kt_tiles):
                    ps_v = pp.tile([128, 512], F32, name="ps_v", tag="pp")
                    for kc in range(4):
                        nc.tensor.matmul(
                            ps_v[:],
                            kt_sb[:, kc * NKP + vt * 128 : kc * NKP + (vt + 1) * 128],
                            wv_sb[:, kc * D : (kc + 1) * D],
                            start=(kc == 0),
                            stop=False,
                        )
                    nc.tensor.matmul(
                        ps_v[:],
                        kt1_sb[0:1, vt * 128 : (vt + 1) * 128],
                        wv1_sb[0:1, :],
                        start=False,
                        stop=True,
                    )
                    v_blk = v_sb[:, vt * 520 : (vt + 1) * 520].rearrange(
                        "p (hh x) -> p hh x", hh=8
                    )
                    s_blk = ps_v.rearrange("p (hh x) -> p hh x", hh=8)
                    nc.scalar.copy(v_blk[:, :, 0:64], s_blk[:, :, :])

            # ---- phase 2: attention ------------------------------------------
            # Per-(head, token) softmax denominator: reciprocal on the ACT
            # table (in place at partition 64, so partition bases match),
            # then PE-broadcast across the 64 head features.
            o_sb = ap.tile([128, 4 * NQ], MM, name="o_sb", tag="big", bufs=2)
            # base-0 copy of q lanes 64..127 for odd heads (engine ops cannot
            # mix partition bases; DMA can move across partitions)
            q_lo = ap.tile([64, 4 * NQ], MM, name="q_lo")
            nc.gpsimd.dma_start(q_lo[:], q_od[64:128, :])

            def chunk_normalize(h, c, po_h):
                # emitted DURING the next head's score matmuls so the PE
                # does not stall waiting for the ACT-table reciprocal
                pr, rh = h // 2, (h % 2) * 64
                pb = pp.tile([64, 512], F32, name="pb", tag="pp")
                nc.tensor.matmul(
                    pb[:], ones_sb[64:65, 0:64], po_h[64:65, :],
                    start=True, stop=True,
                )
                avn = sm.tile([64, 512], MM, name="avn", tag="avn")
                nc.vector.tensor_mul(avn[:, :], po_h[0:64, :], pb[:])
                qsl = slice(pr * NQ + c * 512, pr * NQ + (c + 1) * 512)
                if rh == 0:
                    nc.vector.tensor_add(
                        o_sb[0:64, qsl], avn[:, :], q_ev[0:64, qsl]
                    )
                else:
                    # odd head: build at base 0, then shift to lanes
                    # 64..127 via SBUF-to-SBUF DMA on the Pool queue
                    opre = sm.tile([64, 512], MM, name="opre", tag="opre")
                    nc.vector.tensor_add(opre[:, :], avn[:, :], q_lo[:, qsl])
                    nc.gpsimd.dma_start(o_sb[64:128, qsl], opre[:, :])

            pending = []
            raw = []

            def emit_scores(h):
                pr, rh = h // 2, (h % 2) * 64
                at_tiles = []
                qz = q_od if rh else q_ev
                for i in range(kt_tiles):
                    ps_s = pp.tile([128, NQ], F32, name="ps_s", tag="ps")
                    for c in range(2):
                        nc.tensor.matmul(
                            ps_s[:, c * 512 : (c + 1) * 512],
                            k_sb[:, pr * NKP + i * 128 : pr * NKP + (i + 1) * 128],
                            qz[:, pr * NQ + c * 512 : pr * NQ + (c + 1) * 512],
                            start=True,
                            stop=True,
                        )
                    at_sb = ap.tile([128, NQ], MM, name="at_sb", tag="at",
                                    bufs=21)
                    at_tiles.append(at_sb)
                    nc.scalar.activation(
                        at_sb[:, :], ps_s[:, :], AF.Exp,
                        bias=mb_sb[:, i : i + 1], scale=0.125,
                    )
                return at_tiles

            def emit_av(h, at_tiles):
                for c in range(2):
                    po = pp.tile([65, 512], F32, name="po", tag="po")
                    for i in range(kt_tiles):
                        nc.tensor.matmul(
                            po[:],
                            v_sb[:, i * 520 + h * 65 : i * 520 + (h + 1) * 65],
                            at_tiles[i][:, c * 512 : (c + 1) * 512],
                            start=(i == 0),
                            stop=(i == kt_tiles - 1),
                        )
                    # evict immediately: frees the PSUM bank for the next AV
                    # chain; reciprocal + normalize then run from SBUF
                    po_h = sm.tile([65, 512], MM, name="po_h", tag="poh",
                                   bufs=16)
                    nc.vector.tensor_copy(po_h[:, :], po[:, :])
                    raw.append((h, c, po_h))

            def emit_recips():
                # one Recip table residency covers every queued denominator
                for h, c, po_h in raw:
                    _act_raw(nc, po_h[64:65, :], po_h[64:65, :], AF.Reciprocal)
                    pending.append((h, c, po_h))
                raw.clear()

            # all 40 exps run on a single Exp table residency (quad 1's
            # reciprocals wait until quad 2's exps are done); quad 1's
            # normalizes then interleave with quad 2's AV chains
            sq0 = ap.tile([128, 4 * NQ], MM, name="sq0", tag="sqbuf")
            for qi, quad in enumerate([(1, 0, 3, 2), (5, 4, 7, 6)]):
                ats = [emit_scores(h) for h in quad]
                if qi == 1:
                    emit_recips()
                for j, (h, at_t) in enumerate(zip(quad, ats)):
                    emit_av(h, at_t)
                    if qi == 1:
                        for _ in range(2):
                            if pending:
                                chunk_normalize(*pending.pop(0))
                        if j == 1:
                            # heads 0,1 (feature block 0) now normalized:
                            # square for LN0 while the PE continues AV
                            sl = slice(0, NQ)
                            nc.vector.tensor_mul(
                                sq0[:, sl], o_sb[:, sl], o_sb[:, sl]
                            )
                        if j == 3:
                            sl = slice(NQ, 2 * NQ)
                            nc.vector.tensor_mul(
                                sq0[:, sl], o_sb[:, sl], o_sb[:, sl]
                            )
            emit_recips()
            while pending:
                chunk_normalize(*pending.pop(0))
            # prefetch the Rsqrt ACT table while the PE drains the last AV
            warm = sm.tile([1, 1], F32, name="warm", tag="warm", bufs=1)
            nc.vector.memset(warm[:], 1.0)
            _act_raw(nc, warm[:], warm[:], AF.Rsqrt)

            # ---- layernorm helper --------------------------------------------
            def layer_norm(x_sb, gcol, bcol, out_sb, dma_out=None,
                           sq_pre=None, sq_done=()):
                ones_x = ones_f32 if x_sb.dtype == F32 else ones128
                if sq_pre is not None:
                    sq = sq_pre
                else:
                    sq = ap.tile([128, 4 * NQ], MM, name="sq", tag="kmm_sq")
                for t in range(4):
                    if t in sq_done:
                        continue
                    sl = slice(t * NQ, (t + 1) * NQ)
                    nc.vector.tensor_mul(sq[:, sl], x_sb[:, sl], x_sb[:, sl])
                # both chunks' feature-sum chains first: c1's matmuls keep
                # the PE busy while c0's stats pipeline (ACT/DVE) drains
                # M=16 (wide ones block) avoids the PE's single-column
                # group penalty; only output row 0 is consumed
                ones16 = ones_f32 if x_sb.dtype == F32 else ones_sb[:, 0:16]
                sums = []
                for c in range(2):
                    tag = "po" if c == 0 else "ps"
                    m = ones16.shape[-1] if hasattr(ones16, "shape") else 16
                    ps_su = pp.tile([16, 512], F32, name="ps_su", tag=tag)
                    ps_sq = pp.tile([16, 512], F32, name="ps_sq", tag=tag)
                    sums.append((ps_su, ps_sq))
                    for t in range(4):
                        sl = slice(t * NQ + c * 512, t * NQ + (c + 1) * 512)
                        nc.tensor.matmul(
                            ps_su[0:16] if x_sb.dtype != F32 else ps_su[0:1],
                            ones16, x_sb[:, sl],
                            start=(t == 0), stop=(t == 3),
                        )
                        nc.tensor.matmul(
                            ps_sq[0:16], ones_sb[:, 0:16], sq[:, sl],
                            start=(t == 0), stop=(t == 3),
                        )
                for c in range(2):
                    csl = slice(c * 512, (c + 1) * 512)
                    ps_su, ps_sq = sums[c]
                    mu = sm.tile([1, 512], F32, name="mu", tag="mu")
                    ex2 = sm.tile([1, 512], F32, name="ex2", tag="ex2")
                    nc.scalar.activation(
                        mu[:], ps_su[0:1, :], AF.Identity, scale=1.0 / D
                    )
                    nc.scalar.activation(
                        ex2[:], ps_sq[0:1, :], AF.Identity, scale=1.0 / D
                    )
                    var = sm.tile([1, 512], F32, name="var", tag="var")
                    nc.vector.tensor_mul(var[:], mu[:], mu[:])
                    nc.vector.tensor_sub(var[:], ex2[:], var[:])
                    rstd = sm.tile([1, 512], F32, name="rstd", tag="rstd")
                    _act_raw(nc, rstd[:], var[:], AF.Rsqrt, bias=EPS)
                    rstd_h = sm.tile([1, 512], MM, name="rstd_h", tag="rstdh")
                    mur_h = sm.tile([1, 512], MM, name="mur_h", tag="murh")
                    nc.vector.tensor_copy(rstd_h[:], rstd[:])
                    nc.vector.tensor_mul(mur_h[:], mu[:], rstd[:])
                    pb1 = pp.tile([128, 512], F32, name="pb1", tag="pp")
                    nc.tensor.matmul(
                        pb1[:], ones_sb[0:1, :], rstd_h[0:1, :],
                        start=True, stop=True,
                    )
                    pb2 = pp.tile([128, 512], F32, name="pb2", tag="pp")
                    nc.tensor.matmul(
                        pb2[:], ones_sb[0:1, :], mur_h[0:1, :],
                        start=True, stop=True,
                    )
                    # evict broadcasts to fp16 SBUF on ACT: frees the PSUM
                    # ring at once and lets the hot loop run at fp16 rate
                    rep_r = sm.tile([128, 512], MM, name="rep_r", tag="repr")
                    rep_m = sm.tile([128, 512], MM, name="rep_m", tag="repm")
                    nc.scalar.copy(rep_r[:], pb1[:])
                    nc.scalar.copy(rep_m[:], pb2[:])
                    for t in range(4):
                        sl = slice(t * NQ + c * 512, t * NQ + (c + 1) * 512)
                        tmp = sm.tile([128, 512], MM, name="lntmp", tag="lntmp")
                        nc.vector.tensor_mul(tmp[:], x_sb[:, sl], rep_r[:])
                        nc.vector.tensor_sub(out_sb[:, sl], tmp[:], rep_m[:])
                        if not gb_trivial:
                            nc.vector.tensor_scalar(
                                out_sb[:, sl], out_sb[:, sl],
                                gb_sb[:, gcol + t : gcol + t + 1],
                                gb_sb[:, bcol + t : bcol + t + 1],
                                mult, add,
                            )
                        if dma_out is not None:
                            nc.sync.dma_start(
                                dma_out[t * 128 : (t + 1) * 128,
                                        c * 512 : (c + 1) * 512],
                                out_sb[:, sl],
                            )

            # ---- phase 3: LN0 -------------------------------------------------
            ot0 = ap.tile([128, 4 * NQ], MM, name="ot0", tag="big", bufs=2)
            layer_norm(o_sb, 0, 4, ot0, sq_pre=sq0, sq_done=(0, 1))

            # ---- phase 4: FC + relu + residual -------------------------------
            o1 = ap.tile([128, 4 * NQ], MM, name="o1", tag="big", bufs=2)
            sq1 = ap.tile([128, 4 * NQ], MM, name="sq1", tag="sqbuf")
            for ot in range(4):
                for c in range(2):
                    ps_f = pp.tile([128, 512], F32, name="ps_f", tag="pp")
                    for ft in range(4):
                        nc.tensor.matmul(
                            ps_f[:],
                            wo_sb[:, ft * D + ot * 128 : ft * D + (ot + 1) * 128],
                            ot0[:, ft * NQ + c * 512 : ft * NQ + (c + 1) * 512],
                            start=(ft == 0),
                            stop=(ft == 3),
                        )
                    rl = sm.tile([128, 512], MM, name="rl", tag="avn")
                    nc.scalar.activation(
                        rl[:], ps_f[:], AF.Relu, bias=bo_sb[:, ot : ot + 1],
                    )
                    sl = slice(ot * NQ + c * 512, ot * NQ + (c + 1) * 512)
                    nc.vector.tensor_add(o1[:, sl], ot0[:, sl], rl[:])
                    if c == 1:
                        # square the completed block now: LN1's stats chain
                        # then starts with no DVE work left in front of it
                        bl = slice(ot * NQ, (ot + 1) * NQ)
                        nc.vector.tensor_mul(sq1[:, bl], o1[:, bl], o1[:, bl])

            # ---- phase 5: LN1 -> out ------------------------------------------
            otout = ap.tile([128, 4 * NQ], MM, name="otout", tag="bigo", bufs=1)
            layer_norm(o1, 8, 12, otout, dma_out=out_d,
                       sq_pre=sq1, sq_done=(0, 1, 2, 3))

    _split_multi_waits(nc)
    return nc


_nc_cache = {}


def _get_nc(kt_tiles=8, gb_trivial=False):
    key = (kt_tiles, gb_trivial)
    if key not in _nc_cache:
        _nc_cache[key] = build_nc(kt_tiles, gb_trivial)
    return _nc_cache[key]


def _kt_tiles_for(mask):
    n = int(max(int((mask[b] != 0).sum()) for b in range(mask.shape[0])))
    return max(1, (n + 127) // 128)


def prep_inputs(Q, K, mask, Wq, bq, Wk, bk, Wv, bv, Wo, bo, g0, b0, g1, b1,
                kt_tiles=None):
    f32 = np.float32
    f16 = np.float16
    ones_h = np.ones((128, 128), f16)
    if kt_tiles is None:
        kt_tiles = _kt_tiles_for(mask)
    nkp = kt_tiles * 128

    def percol(v, dt=f32):  # [512] feature vector -> [128, 4] per-partition
        return np.ascontiguousarray(np.asarray(v, f32).reshape(4, 128).T.astype(dt))

    wv_h = np.ascontiguousarray(
        np.vstack([np.asarray(Wv, f32), np.asarray(bv, f32)[None, :]]).astype(f16)
    )
    gb = np.concatenate([percol(g0), percol(b0), percol(g1), percol(b1)], axis=1)
    wq_h = np.ascontiguousarray(np.asarray(Wq, f32).astype(f16))
    wk_h = np.ascontiguousarray(np.asarray(Wk, f32).astype(f16))
    wo_h = np.ascontiguousarray(np.asarray(Wo, f32).astype(f16))

    in_maps = []
    for b in range(B):
        qt = np.ascontiguousarray(np.asarray(Q[b], f32).T.astype(f16))
        idx = np.nonzero(mask[b] != 0)[0]
        kc = np.zeros((nkp, D), f32)
        kc[: len(idx)] = np.asarray(K[b], f32)[idx]
        kt = np.ascontiguousarray(
            np.vstack([kc.T, np.ones((1, nkp), f32)]).astype(f16)
        )
        mb = np.full(nkp, np.float32(NEG))
        mb[: len(idx)] = 0.0
        mb = np.ascontiguousarray(mb.reshape(kt_tiles, 128).T.astype(f32))
        in_maps.append(
            {
                "qt": qt,
                "kt": kt,
                "wq": wq_h,
                "wk": wk_h,
                "wv": wv_h,
                "wo": wo_h,
                "bq": percol(bq),
                "bk": percol(bk),
                "bo": percol(bo),
                "mb": mb,
                "gb": gb,
                "on": ones_h,
            }
        )
    return in_maps


def _gb_trivial(g0, b0, g1, b1):
    return bool(
        np.all(np.asarray(g0) == 1.0) and np.all(np.asarray(b0) == 0.0)
        and np.all(np.asarray(g1) == 1.0) and np.all(np.asarray(b1) == 0.0)
    )


def kernel(Q, K, mask, Wq, bq, Wk, bk, Wv, bv, Wo, bo, g0, b0, g1, b1):
    mask = np.asarray(mask)
    kt_tiles = _kt_tiles_for(mask)
    nc = _get_nc(kt_tiles, _gb_trivial(g0, b0, g1, b1))
    in_maps = prep_inputs(
        Q, K, mask, Wq, bq, Wk, bk, Wv, bv, Wo, bo, g0, b0, g1, b1, kt_tiles
    )
    res = run_bass_kernel_spmd(nc, in_maps, list(range(N_CORES)))
    out = np.stack(
        [np.ascontiguousarray(res.results[i]["out"].T) for i in range(N_CORES)]
    )
    return out.astype(np.float32)


# revision 45
# speedup vs baseline: 1.0247x; 1.0247x over previous
"""Trainium2 Bass kernel for the masked-attention block (nn_MAB_61607010894006).

Sharding: data-parallel over batch B=8 across 8 NeuronCores (one batch row
per core, weights replicated, no collectives).

Per-core strategy: activations live transposed ("feature-major",
[features, tokens]); all matmul operands are float16 (full PE rate; the
fp32/fp32r path runs in fp32_mode=HIGH at half clock). PSUM accumulation is
fp32 throughout, evictions round to fp16.

  qT/kT      = W.T @ X.T      (lhsT = W chunk, rhs = XT chunk)
  S^T        = kT_h' @ qT_h   (k tokens on partitions, q tokens free)
  softmax    : exp on ScalarE with mask as per-partition bias (-1e9), no
               max-subtraction (scores are O(1)); normalization deferred:
  o^T        = [v | 1]' @ A^T accumulated over k tiles -> row 64 is the
               softmax denominator; the PSUM result is evicted to SBUF at
               once (freeing the bank for the next head), the denominator
               row inverted in place on the ACT table, PE-broadcast across
               the 64 head lanes, then normalize+residual on DVE. The
               broadcast+normalize of head h is emitted in the middle of
               head h+1's score matmuls so the PE never waits on the ACT
               reciprocal (table swap ~1.3us). Odd heads assemble at
               partition base 0 and are shifted to lanes 64..127 by a
               Pool-queue SBUF DMA (engine ops cannot cross bases).
  layernorm  : feature-dim (partition) sums via ones-column matmuls on PE;
               per-token stats broadcast back via tiny PE ones-matmuls.
  FC         = Wo' @ OT, bias+relu fused into the ScalarE eviction.

Mask compaction: only unmasked key tokens are shipped per core (masked ones
contribute exactly +0.0 to the softmax numerator and denominator), padded
to a 128 multiple.
"""

import sys

sys.path.insert(0, "/opt/trn_rl_repo")

import numpy as np

import concourse.bass as bass
import concourse.mybir as mybir
import concourse.tile as tile
from concourse.bass_utils import run_bass_kernel_spmd

F32 = mybir.dt.float32
F16 = mybir.dt.float16
AF = mybir.ActivationFunctionType

B, NQ, NK, D, H, DH = 8, 1024, 1024, 512, 8, 64
EPS = 1e-5
NEG = -1e9
N_CORES = 8

MM = F16


def _split_multi_waits(nc):
    """This toolchain's walrus allows ONE sem wait per TPB instruction; Tile
    can emit several (kernel-tail drain). Hoist extras onto preceding
    single-wait NOPs on the same engine stream (equivalent: in-order issue).
    """
    multi_update = []
    for fn in nc.m.functions:
        for bb in fn.blocks:
            insts = bb.instructions
            new = []
            changed = False
            for inst in insts:
                si = inst.sync_info
                if si is not None and si.on_wait and len(si.on_wait) > 1:
                    waits = list(si.on_wait)
                    for w in waits[:-1]:
                        nop = mybir.InstNoOp(
                            name=f"I-wsplit-{nc.next_id()}", engine=inst.engine
                        )
                        nop.sync_info = mybir.SyncInfo(on_wait=[w], on_update=[])
                        new.append(nop)
                    inst.sync_info = mybir.SyncInfo(
                        on_wait=[waits[-1]], on_update=list(si.on_update)
                    )
                    changed = True
                if si is not None and si.on_update and len(si.on_update) > 1:
                    multi_update.append(inst.name)
                new.append(inst)
            if changed:
                bb.instructions = new
    if multi_update:
        raise RuntimeError(f">1 sem update unsupported: {multi_update[:10]}")


def _act_raw(nc, out, in_, func, bias=0.0, scale=1.0):
    """Raw InstActivation (bypasses the bass Reciprocal/Rsqrt accuracy guard;
    measured ~4e-5 max rel err on HW, inside this kernel's error budget)."""
    eng = nc.scalar
    inputs = [eng.lower_ap(in_)]
    for arg in (bias, scale, 0.0):
        inputs.append(mybir.ImmediateValue(dtype=mybir.dt.float32, value=arg))
    return eng.add_instruction(
        mybir.InstActivation(
            name=f"I-actraw-{nc.next_id()}",
            func=func,
            ins=inputs,
            outs=[eng.lower_ap(out)],
        )
    )


def chunks(n, w=512):
    out, s = [], 0
    while s < n:
        out.append((s, min(w, n - s)))
        s += min(w, n - s)
    return out


def build_nc(kt_tiles=8, gb_trivial=False):
    NKP = kt_tiles * 128  # compacted+padded key/value token count
    nc = bass.Bass()

    qt_d = nc.dram_tensor("qt", [D, NQ], MM, kind="ExternalInput")
    kt_d = nc.dram_tensor("kt", [D + 1, NKP], MM, kind="ExternalInput")  # +ones
    wq_d = nc.dram_tensor("wq", [D, D], MM, kind="ExternalInput")
    wk_d = nc.dram_tensor("wk", [D, D], MM, kind="ExternalInput")
    wv_d = nc.dram_tensor("wv", [D + 1, D], MM, kind="ExternalInput")  # +bv row
    wo_d = nc.dram_tensor("wo", [D, D], MM, kind="ExternalInput")
    bq_d = nc.dram_tensor("bq", [128, 4], F32, kind="ExternalInput")
    bk_d = nc.dram_tensor("bk", [128, 4], F32, kind="ExternalInput")
    bo_d = nc.dram_tensor("bo", [128, 4], F32, kind="ExternalInput")
    mb_d = nc.dram_tensor("mb", [128, kt_tiles], F32, kind="ExternalInput")
    gb_d = nc.dram_tensor("gb", [128, 16], F32, kind="ExternalInput")  # g0b0g1b1
    on_d = nc.dram_tensor("on", [128, 128], MM, kind="ExternalInput")  # all ones
    out_d = nc.dram_tensor("out", [D, NQ], MM, kind="ExternalOutput")

    mult, add = mybir.AluOpType.mult, mybir.AluOpType.add

    with tile.TileContext(nc) as tc:
        with (
            tc.tile_pool(name="wp", bufs=1) as wp,
            tc.tile_pool(name="ap", bufs=1) as ap,
            tc.tile_pool(name="sm", bufs=2) as sm,
            tc.tile_pool(name="pp", bufs=2, space="PSUM") as pp,
        ):
            # ---- weights (one batched DMA each; wo deferred to the end of
            # the issue stream since it is only needed at phase 4) ----------
            wq_sb = wp.tile([128, 4 * D], MM, name="wq_sb")
            wk_sb = wp.tile([128, 4 * D], MM, name="wk_sb")
            wv_sb = wp.tile([128, 4 * D], MM, name="wv_sb")
            wv1_sb = wp.tile([1, D], MM, name="wv1_sb")
            wo_sb = wp.tile([128, 4 * D], MM, name="wo_sb")
            bq_sb = wp.tile([128, 4], F32, name="bq_sb")
            bk_sb = wp.tile([128, 4], F32, name="bk_sb")
            bo_sb = wp.tile([128, 4], F32, name="bo_sb")
            mb_sb = wp.tile([128, kt_tiles], F32, name="mb_sb")
            gb_sb = wp.tile([128, 16], F32, name="gb_sb")
            ones_sb = wp.tile([128, 128], MM, name="ones_sb")
            kt1_sb = wp.tile([1, NKP], MM, name="kt1_sb")

            def load4(dst, src, n):
                # dst [128, 4*n] <- src [4*128, n] as one strided DMA
                nc.sync.dma_start(
                    dst.rearrange("p (t n) -> p t n", t=4),
                    src.rearrange("(t p) n -> p t n", p=128),
                )

            # ---- staging (released after phase 1) ----------------------------
            with tc.tile_pool(name="stg", bufs=1) as stg:
                qt_sb = stg.tile([128, 4 * NQ], MM, name="qt_sb")
                kt_sb = stg.tile([128, 4 * NKP], MM, name="kt_sb")

                # single DMA engine services all queues: use ONE queue in
                # strict priority order (consumers first), qt/kt chunked so
                # the first projection starts after ~0.75MB
                nc.sync.dma_start(bq_sb[:], bq_d[:])
                # first operands split across all three DMA-capable queues so
                # descriptor generation and transfer overlap
                qs = [nc.sync, nc.scalar, nc.gpsimd]
                for hc in range(2):
                    for kc in range(4):
                        if hc == 0:
                            qs[kc % 3].dma_start(
                                wq_sb[:, kc * D : (kc + 1) * D],
                                wq_d[kc * 128 : (kc + 1) * 128, :],
                            )
                        qs[(kc * 2 + hc + 1) % 3].dma_start(
                            qt_sb[:, kc * NQ + hc * 512 : kc * NQ + (hc + 1) * 512],
                            qt_d[kc * 128 : (kc + 1) * 128,
                                 hc * 512 : (hc + 1) * 512],
                        )
                nc.sync.dma_start(bk_sb[:], bk_d[:])
                for kc in range(4):
                    nc.sync.dma_start(
                        wk_sb[:, kc * D : (kc + 1) * D],
                        wk_d[kc * 128 : (kc + 1) * 128, :],
                    )
                    nc.sync.dma_start(
                        kt_sb[:, kc * NKP : kc * NKP + 512],
                        kt_d[kc * 128 : (kc + 1) * 128, 0:512],
                    )
                for kc in range(4):
                    nc.sync.dma_start(
                        kt_sb[:, kc * NKP + 512 : (kc + 1) * NKP],
                        kt_d[kc * 128 : (kc + 1) * 128, 512:NKP],
                    )
                nc.sync.dma_start(
                    wv_sb.rearrange("p (t n) -> p t n", t=4),
                    wv_d[0:D, :].rearrange("(t p) n -> p t n", p=128),
                )
                nc.sync.dma_start(wv1_sb[:, :], wv_d[D : D + 1, :])
                nc.sync.dma_start(kt1_sb[:, :], kt_d[D : D + 1, :])
                nc.sync.dma_start(mb_sb[:], mb_d[:])
                nc.sync.dma_start(ones_sb[:], on_d[:])
                nc.sync.dma_start(gb_sb[:], gb_d[:])
                nc.sync.dma_start(bo_sb[:], bo_d[:])
                nc.sync.dma_start(
                    wo_sb.rearrange("p (t n) -> p t n", t=4),
                    wo_d.rearrange("(t p) n -> p t n", p=128),
                )

                ones128 = ones_sb[:, 0:1]
                ones_f32 = wp.tile([128, 1], F32, name="ones_f32")
                nc.vector.memset(ones_f32[:], 1.0)

                # ---- persistent activations ------------------------------
                # q kept as two parity copies with the other head's rows
                # zeroed: scores then contract over the full 128-row head
                # pair (zeros kill the other head), avoiding the PE's
                # 64-row row-group switching penalty
                q_ev = ap.tile([128, 4 * NQ], MM, name="q_ev")
                q_od = ap.tile([128, 4 * NQ], MM, name="q_od")
                nc.vector.memset(q_ev[64:128, :], 0.0)
                nc.vector.memset(q_od[0:64, :], 0.0)
                k_sb = ap.tile([128, 4 * NKP], MM, name="k_sb", tag="kmm_sq")
                # v: per k-tile, 8 heads of [v(64)|1]
                v_sb = ap.tile([128, kt_tiles * 520], MM, name="v_sb")
                v_ones = v_sb.rearrange(
                    "p (i hh x) -> p i hh x", i=kt_tiles, hh=8
                )[:, :, :, 64]
                nc.vector.memset(v_ones, 1.0)

                # ---- phase 1: projections (all Q first: the Q operands are
                # the first 1.5MB to land, K/V stream in behind) -----------
                for cs, cw in chunks(NQ):
                    for t in range(4):
                        ps_q = pp.tile([128, 512], F32, name="ps_q", tag="pp")
                        for kc in range(4):
                            nc.tensor.matmul(
                                ps_q[:, 0:cw],
                                wq_sb[:, kc * D + t * 128 : kc * D + (t + 1) * 128],
                                qt_sb[:, kc * NQ + cs : kc * NQ + cs + cw],
                                start=(kc == 0),
                                stop=(kc == 3),
                            )
                        dst = slice(t * NQ + cs, t * NQ + cs + cw)
                        nc.scalar.activation(
                            q_ev[0:64, dst], ps_q[0:64, 0:cw], AF.Identity,
                            bias=bq_sb[0:64, t : t + 1],
                        )
                        nc.scalar.activation(
                            q_od[64:128, dst], ps_q[64:128, 0:cw], AF.Identity,
                            bias=bq_sb[64:128, t : t + 1],
                        )
                for cs, cw in chunks(NKP):
                    for t in range(4):
                        ps_k = pp.tile([128, 512], F32, name="ps_k", tag="pp")
                        for kc in range(4):
                            nc.tensor.matmul(
                                ps_k[:, 0:cw],
                                wk_sb[:, kc * D + t * 128 : kc * D + (t + 1) * 128],
                                kt_sb[:, kc * NKP + cs : kc * NKP + cs + cw],
                                start=(kc == 0),
                                stop=(kc == 3),
                            )
                        dst = slice(t * NKP + cs, t * NKP + cs + cw)
                        nc.scalar.activation(
                            k_sb[:, dst], ps_k[:, 0:cw], AF.Identity,
                            bias=bk_sb[:, t : t + 1],
                        )

                # v token-major [NKP, 512] (+bias via augmented ones row)
                for vt in range(kt_tiles):
                    ps_v = pp.tile([128, 512], F32, name="ps_v", tag="pp")
                    for kc in range(4):
                        nc.tensor.matmul(
                            ps_v[:],
                            kt_sb[:, kc * NKP + vt * 128 : kc * NKP + (vt + 1) * 128],
                            wv_sb[:, kc * D : (kc + 1) * D],
                            start=(kc == 0),
                            stop=False,
                        )
                    nc.tensor.matmul(
                        ps_v[:],
                        kt1_sb[0:1, vt * 128 : (vt + 1) * 128],
                        wv1_sb[0:1, :],
                        start=False,
                        stop=True,
                    )
                    v_blk = v_sb[:, vt * 520 : (vt + 1) * 520].rearrange(
                        "p (hh x) -> p hh x", hh=8
                    )
                    s_blk = ps_v.rearrange("p (hh x) -> p hh x", hh=8)
                    nc.scalar.copy(v_blk[:, :, 0:64], s_blk[:, :, :])

            # ---- phase 2: attention ------------------------------------------
            # Per-(head, token) softmax denominator: reciprocal on the ACT
            # table (in place at partition 64, so partition bases match),
            # then PE-broadcast across the 64 head features.
            o_sb = ap.tile([128, 4 * NQ], MM, name="o_sb", tag="big", bufs=2)
            # base-0 copy of q lanes 64..127 for odd heads (engine ops cannot
            # mix partition bases; DMA can move across partitions)
            q_lo = ap.tile([64, 4 * NQ], MM, name="q_lo")
            nc.gpsimd.dma_start(q_lo[:], q_od[64:128, :])

            def chunk_normalize(h, c, po_h):
                # emitted DURING the next head's score matmuls so the PE
                # does not stall waiting for the ACT-table reciprocal
                pr, rh = h // 2, (h % 2) * 64
                pb = pp.tile([64, 512], F32, name="pb", tag="pp")
                nc.tensor.matmul(
                    pb[:], ones_sb[64:65, 0:64], po_h[64:65, :],
                    start=True, stop=True,
                )
                avn = sm.tile([64, 512], MM, name="avn", tag="avn")
                nc.vector.tensor_mul(avn[:, :], po_h[0:64, :], pb[:])
                qsl = slice(pr * NQ + c * 512, pr * NQ + (c + 1) * 512)
                if rh == 0:
                    nc.vector.tensor_add(
                        o_sb[0:64, qsl], avn[:, :], q_ev[0:64, qsl]
                    )
                else:
                    # odd head: build at base 0, then shift to lanes
                    # 64..127 via SBUF-to-SBUF DMA on the Pool queue
                    opre = sm.tile([64, 512], MM, name="opre", tag="opre")
                    nc.vector.tensor_add(opre[:, :], avn[:, :], q_lo[:, qsl])
                    nc.gpsimd.dma_start(o_sb[64:128, qsl], opre[:, :])

            pending = []

            def emit_scores(h):
                pr, rh = h // 2, (h % 2) * 64
                at_tiles = []
                for i in range(kt_tiles):
                    ps_s = pp.tile([128, NQ], F32, name="ps_s", tag="ps")
                    qz = q_od if rh else q_ev
                    for c in range(2):
                        nc.tensor.matmul(
                            ps_s[:, c * 512 : (c + 1) * 512],
                            k_sb[:, pr * NKP + i * 128 : pr * NKP + (i + 1) * 128],
                            qz[:, pr * NQ + c * 512 : pr * NQ + (c + 1) * 512],
                            start=True,
                            stop=True,
                        )
                    at_sb = ap.tile([128, NQ], MM, name="at_sb", tag="at",
                                    bufs=21)
                    at_tiles.append(at_sb)
                    nc.scalar.activation(
                        at_sb[:, :], ps_s[:, :], AF.Exp,
                        bias=mb_sb[:, i : i + 1], scale=0.125,
                    )
                    if pending and i >= min(2, kt_tiles - 1):
                        chunk_normalize(*pending.pop(0))
                return at_tiles

            def emit_av(h, at_tiles):
                for c in range(2):
                    po = pp.tile([65, 512], F32, name="po", tag="po")
                    for i in range(kt_tiles):
                        nc.tensor.matmul(
                            po[:],
                            v_sb[:, i * 520 + h * 65 : i * 520 + (h + 1) * 65],
                            at_tiles[i][:, c * 512 : (c + 1) * 512],
                            start=(i == 0),
                            stop=(i == kt_tiles - 1),
                        )
                    # evict immediately: frees the PSUM bank for the next AV
                    # chain; reciprocal + normalize then run from SBUF
                    po_h = sm.tile([65, 512], MM, name="po_h", tag="poh",
                                   bufs=16)
                    nc.vector.tensor_copy(po_h[:, :], po[:, :])
                    _act_raw(nc, po_h[64:65, :], po_h[64:65, :], AF.Reciprocal)
                    pending.append((h, c, po_h))

            # heads in pairs: both heads' exps run on one Exp table
            # residency, then all 4 reciprocals on one Recip residency
            # (halves the ACT table ping-pong); odd heads first so the
            # final head needs no Pool-DMA lane shift before LN0
            sq0 = ap.tile([128, 4 * NQ], MM, name="sq0", tag="sqbuf")
            for qi, quad in enumerate([(1, 0, 3, 2), (5, 4, 7, 6)]):
                ats = [emit_scores(h) for h in quad]
                while pending:
                    chunk_normalize(*pending.pop(0))
                if qi == 1:
                    # feature blocks 0,1 (heads 0..3) are complete: square
                    # them for LN0 while the PE runs this quad's AV chains
                    for t in (0, 1):
                        sl = slice(t * NQ, (t + 1) * NQ)
                        nc.vector.tensor_mul(
                            sq0[:, sl], o_sb[:, sl], o_sb[:, sl]
                        )
                for h, at_t in zip(quad, ats):
                    emit_av(h, at_t)
            while pending:
                chunk_normalize(*pending.pop(0))
            # prefetch the Rsqrt ACT table while the PE drains the last AV
            warm = sm.tile([1, 1], F32, name="warm", tag="warm", bufs=1)
            nc.vector.memset(warm[:], 1.0)
            _act_raw(nc, warm[:], warm[:], AF.Rsqrt)

            # ---- layernorm helper --------------------------------------------
            def layer_norm(x_sb, gcol, bcol, out_sb, dma_out=None,
                           sq_pre=None, sq_done=()):
                ones_x = ones_f32 if x_sb.dtype == F32 else ones128
                if sq_pre is not None:
                    sq = sq_pre
                else:
                    sq = ap.tile([128, 4 * NQ], MM, name="sq", tag="kmm_sq")
                for t in range(4):
                    if t in sq_done:
                        continue
                    sl = slice(t * NQ, (t + 1) * NQ)
                    nc.vector.tensor_mul(sq[:, sl], x_sb[:, sl], x_sb[:, sl])
                # both chunks' feature-sum chains first: c1's matmuls keep
                # the PE busy while c0's stats pipeline (ACT/DVE) drains
                # M=16 (wide ones block) avoids the PE's single-column
                # group penalty; only output row 0 is consumed
                ones16 = ones_f32 if x_sb.dtype == F32 else ones_sb[:, 0:16]
                sums = []
                for c in range(2):
                    tag = "po" if c == 0 else "ps"
                    m = ones16.shape[-1] if hasattr(ones16, "shape") else 16
                    ps_su = pp.tile([16, 512], F32, name="ps_su", tag=tag)
                    ps_sq = pp.tile([16, 512], F32, name="ps_sq", tag=tag)
                    sums.append((ps_su, ps_sq))
                    for t in range(4):
                        sl = slice(t * NQ + c * 512, t * NQ + (c + 1) * 512)
                        nc.tensor.matmul(
                            ps_su[0:16] if x_sb.dtype != F32 else ps_su[0:1],
                            ones16, x_sb[:, sl],
                            start=(t == 0), stop=(t == 3),
                        )
                        nc.tensor.matmul(
                            ps_sq[0:16], ones_sb[:, 0:16], sq[:, sl],
                            start=(t == 0), stop=(t == 3),
                        )
                for c in range(2):
                    csl = slice(c * 512, (c + 1) * 512)
                    ps_su, ps_sq = sums[c]
                    mu = sm.tile([1, 512], F32, name="mu", tag="mu")
                    ex2 = sm.tile([1, 512], F32, name="ex2", tag="ex2")
                    nc.scalar.activation(
                        mu[:], ps_su[0:1, :], AF.Identity, scale=1.0 / D
                    )
                    nc.scalar.activation(
                        ex2[:], ps_sq[0:1, :], AF.Identity, scale=1.0 / D
                    )
                    var = sm.tile([1, 512], F32, name="var", tag="var")
                    nc.vector.tensor_mul(var[:], mu[:], mu[:])
                    nc.vector.tensor_sub(var[:], ex2[:], var[:])
                    rstd = sm.tile([1, 512], F32, name="rstd", tag="rstd")
                    _act_raw(nc, rstd[:], var[:], AF.Rsqrt, bias=EPS)
                    rstd_h = sm.tile([1, 512], MM, name="rstd_h", tag="rstdh")
                    mur_h = sm.tile([1, 512], MM, name="mur_h", tag="murh")
                    nc.vector.tensor_copy(rstd_h[:], rstd[:])
                    nc.vector.tensor_mul(mur_h[:], mu[:], rstd[:])
                    pb1 = pp.tile([128, 512], F32, name="pb1", tag="pp")
                    nc.tensor.matmul(
                        pb1[:], ones_sb[0:1, :], rstd_h[0:1, :],
                        start=True, stop=True,
                    )
                    pb2 = pp.tile([128, 512], F32, name="pb2", tag="pp")
                    nc.tensor.matmul(
                        pb2[:], ones_sb[0:1, :], mur_h[0:1, :],
                        start=True, stop=True,
                    )
                    # evict broadcasts to fp16 SBUF on ACT: frees the PSUM
                    # ring at once and lets the hot loop run at fp16 rate
                    rep_r = sm.tile([128, 512], MM, name="rep_r", tag="repr")
                    rep_m = sm.tile([128, 512], MM, name="rep_m", tag="repm")
                    nc.scalar.copy(rep_r[:], pb1[:])
                    nc.scalar.copy(rep_m[:], pb2[:])
                    for t in range(4):
                        sl = slice(t * NQ + c * 512, t * NQ + (c + 1) * 512)
                        tmp = sm.tile([128, 512], MM, name="lntmp", tag="lntmp")
                        nc.vector.tensor_mul(tmp[:], x_sb[:, sl], rep_r[:])
                        nc.vector.tensor_sub(out_sb[:, sl], tmp[:], rep_m[:])
                        if not gb_trivial:
                            nc.vector.tensor_scalar(
                                out_sb[:, sl], out_sb[:, sl],
                                gb_sb[:, gcol + t : gcol + t + 1],
                                gb_sb[:, bcol + t : bcol + t + 1],
                                mult, add,
                            )
                        if dma_out is not None:
                            nc.sync.dma_start(
                                dma_out[t * 128 : (t + 1) * 128,
                                        c * 512 : (c + 1) * 512],
                                out_sb[:, sl],
                            )

            # ---- phase 3: LN0 -------------------------------------------------
            ot0 = ap.tile([128, 4 * NQ], MM, name="ot0", tag="big", bufs=2)
            layer_norm(o_sb, 0, 4, ot0, sq_pre=sq0, sq_done=(0, 1))

            # ---- phase 4: FC + relu + residual -------------------------------
            o1 = ap.tile([128, 4 * NQ], MM, name="o1", tag="big", bufs=2)
            sq1 = ap.tile([128, 4 * NQ], MM, name="sq1", tag="sqbuf")
            for ot in range(4):
                for c in range(2):
                    ps_f = pp.tile([128, 512], F32, name="ps_f", tag="pp")
                    for ft in range(4):
                        nc.tensor.matmul(
                            ps_f[:],
                            wo_sb[:, ft * D + ot * 128 : ft * D + (ot + 1) * 128],
                            ot0[:, ft * NQ + c * 512 : ft * NQ + (c + 1) * 512],
                            start=(ft == 0),
                            stop=(ft == 3),
                        )
                    rl = sm.tile([128, 512], MM, name="rl", tag="avn")
                    nc.scalar.activation(
                        rl[:], ps_f[:], AF.Relu, bias=bo_sb[:, ot : ot + 1],
                    )
                    sl = slice(ot * NQ + c * 512, ot * NQ + (c + 1) * 512)
                    nc.vector.tensor_add(o1[:, sl], ot0[:, sl], rl[:])
                    if c == 1:
                        # square the completed block now: LN1's stats chain
                        # then starts with no DVE work left in front of it
                        bl = slice(ot * NQ, (ot + 1) * NQ)
                        nc.vector.tensor_mul(sq1[:, bl], o1[:, bl], o1[:, bl])

            # ---- phase 5: LN1 -> out ------------------------------------------
            otout = ap.tile([128, 4 * NQ], MM, name="otout", tag="bigo", bufs=1)
            layer_norm(o1, 8, 12, otout, dma_out=out_d,
                       sq_pre=sq1, sq_done=(0, 1, 2, 3))

    _split_multi_waits(nc)
    return nc


_nc_cache = {}


def _get_nc(kt_tiles=8, gb_trivial=False):
    key = (kt_tiles, gb_trivial)
    if key not in _nc_cache:
        _nc_cache[key] = build_nc(kt_tiles, gb_trivial)
    return _nc_cache[key]


def _kt_tiles_for(mask):
    n = int(max(int((mask[b] != 0).sum()) for b in range(mask.shape[0])))
    return max(1, (n + 127) // 128)


def prep_inputs(Q, K, mask, Wq, bq, Wk, bk, Wv, bv, Wo, bo, g0, b0, g1, b1,
                kt_tiles=None):
    f32 = np.float32
    f16 = np.float16
    ones_h = np.ones((128, 128), f16)
    if kt_tiles is None:
        kt_tiles = _kt_tiles_for(mask)
    nkp = kt_tiles * 128

    def percol(v, dt=f32):  # [512] feature vector -> [128, 4] per-partition
        return np.ascontiguousarray(np.asarray(v, f32).reshape(4, 128).T.astype(dt))

    wv_h = np.ascontiguousarray(
        np.vstack([np.asarray(Wv, f32), np.asarray(bv, f32)[None, :]]).astype(f16)
    )
    gb = np.concatenate([percol(g0), percol(b0), percol(g1), percol(b1)], axis=1)
    wq_h = np.ascontiguousarray(np.asarray(Wq, f32).astype(f16))
    wk_h = np.ascontiguousarray(np.asarray(Wk, f32).astype(f16))
    wo_h = np.ascontiguousarray(np.asarray(Wo, f32).astype(f16))

    in_maps = []
    for b in range(B):
        qt = np.ascontiguousarray(np.asarray(Q[b], f32).T.astype(f16))
        idx = np.nonzero(mask[b] != 0)[0]
        kc = np.zeros((nkp, D), f32)
        kc[: len(idx)] = np.asarray(K[b], f32)[idx]
        kt = np.ascontiguousarray(
            np.vstack([kc.T, np.ones((1, nkp), f32)]).astype(f16)
        )
        mb = np.full(nkp, np.float32(NEG))
        mb[: len(idx)] = 0.0
        mb = np.ascontiguousarray(mb.reshape(kt_tiles, 128).T.astype(f32))
        in_maps.append(
            {
                "qt": qt,
                "kt": kt,
                "wq": wq_h,
                "wk": wk_h,
                "wv": wv_h,
                "wo": wo_h,
                "bq": percol(bq),
                "bk": percol(bk),
                "bo": percol(bo),
                "mb": mb,
                "gb": gb,
                "on": ones_h,
            }
        )
    return in_maps


def _gb_trivial(g0, b0, g1, b1):
    return bool(
        np.all(np.asarray(g0) == 1.0) and np.all(np.asarray(b0) == 0.0)
        and np.all(np.asarray(g1) == 1.0) and np.all(np.asarray(b1) == 0.0)
    )


def kernel(Q, K, mask, Wq, bq, Wk, bk, Wv, bv, Wo, bo, g0, b0, g1, b1):
    mask = np.asarray(mask)
    kt_tiles = _kt_tiles_for(mask)
    nc = _get_nc(kt_tiles, _gb_trivial(g0, b0, g1, b1))
    in_maps = prep_inputs(
        Q, K, mask, Wq, bq, Wk, bk, Wv, bv, Wo, bo, g0, b0, g1, b1, kt_tiles
    )
    res = run_bass_kernel_spmd(nc, in_maps, list(range(N_CORES)))
    out = np.stack(
        [np.ascontiguousarray(res.results[i]["out"].T) for i in range(N_CORES)]
    )
    return out.astype(np.float32)
